# Initial kernel scaffold
#
"""Trainium2 Bass kernel for nn_AxialBlock (gnn_message_passing).

Computation (see reference):
  xg    = x @ W_g^T + b_g                                  (T,E,B,D)
  xmean = xg.mean(0)                                       (E,B,D)
  grid  = pad-filled (N+1,N+1,B,D); grid[n0,n1] = xmean    (last edge wins)
  feats_out = cat(feats, grid[1:,1:]) @ W_lin^T -> @ W_f^T (N,N,B,D)
  x_out = xg + feats_out[g0,g1,bidx]                       (T,E,B,D)

Algebraic restructuring (exact, linearity of the two chained linears):
  Wc = W_f @ W_lin ; Wc1 = Wc[:,:D] ; Wc2 = Wc[:,D:]
  b_comb = W_f @ b_lin + b_f
  feats_out = feats @ Wc1^T + b_comb + (grid-term)
    - non-edge positions: grid row == pad  -> + pad * rowsum(Wc2)   (c0 bias)
    - edge positions (n0-1,n1-1):          -> + ymean[win(pos)]
  ymean = xmean @ Wc2^T = (mean_t xg) @ Wc2^T = xbar @ (Wc2 @ W_g)^T + Wc2 @ b_g
  gather rows x_out needs are exactly the edge-position rows of feats_out.

Sharding over 8 cores: feats grid rows (first N axis) 48 rows/core for the
dense pass + scatter; x is sharded over E (512 edges/core) for xg / x_out.
Edge-dependent rows are computed per-core from host-pre-gathered inputs
(pure indexing on the host; all arithmetic on device).

Device layout trick: all matmul inputs are fed pre-transposed (D on the
partition axis) so the PE consumes them directly as the stationary operand
(out rows land in natural row-major layout for contiguous DMA out).
"""

import numpy as np

T, E, B, D, N = 4, 4096, 2, 128, 384
NCORES = 8
RS = N // NCORES            # feats rows per core (48)
JROWS = RS * N * B          # dense rows per core (36864)
ESH = E // NCORES           # edges per core (512)
GS = ESH * B                # gather slots per core (1024)
F32 = np.float32

_BUILD_CACHE = {}


def _build_bass(slot_s):
    """Build (and cache) the Bass module. slot_s = padded scatter-slot count."""
    key = slot_s
    if key in _BUILD_CACHE:
        return _BUILD_CACHE[key]

    import concourse.bacc as bacc
    import concourse.mybir as mybir
    from concourse import bass
    from concourse.tile import TileContext
    from concourse.tile_rust import add_dep_helper

    f32 = mybir.dt.float32
    i32 = mybir.dt.int32
    nslot = slot_s + GS

    nc = bacc.Bacc("TRN2")
    ft = nc.dram_tensor("ft", (128, JROWS), f32, kind="ExternalInput")
    xt = nc.dram_tensor("xt", (128, T * GS), f32, kind="ExternalInput")
    xu = nc.dram_tensor("xu", (128, T * nslot), f32, kind="ExternalInput")
    fe = nc.dram_tensor("fe", (128, nslot), f32, kind="ExternalInput")
    sc = nc.dram_tensor("sc", (slot_s, 1), i32, kind="ExternalInput")
    wcat = nc.dram_tensor("wcat", (128, 3 * 128), f32, kind="ExternalInput")
    brep = nc.dram_tensor("brep", (128, 3 * 128), f32, kind="ExternalInput")
    out_f = nc.dram_tensor("out_f", (JROWS, 128), f32, kind="ExternalOutput")
    out_x = nc.dram_tensor("out_x", (T * GS, 128), f32, kind="ExternalOutput")

    ts = bass.ts
    CH = 3072               # main-pass chunk: 24 tiles of 128 rows
    NCH = JROWS // CH       # 12 chunks

    with TileContext(nc) as tc:
        with (
            tc.tile_pool(name="const", bufs=1) as cpool,
            tc.tile_pool(name="xside", bufs=1) as xpool,
            tc.tile_pool(name="slots", bufs=1) as spool,
            tc.tile_pool(name="min", bufs=3) as min_pool,
            tc.tile_pool(name="mout", bufs=3) as mout_pool,
            tc.tile_pool(name="psum", bufs=8, space="PSUM") as psum,
        ):
            w_sb = cpool.tile([128, 384], f32)
            nc.sync.dma_start(out=w_sb[:], in_=wcat[:, :])
            b_sb = cpool.tile([128, 384], f32)
            nc.sync.dma_start(out=b_sb[:], in_=brep[:, :])
            wc1t = w_sb[:, 0:128]     # Wc1^T  (d, o)
            wxt = w_sb[:, 128:256]    # (Wx/4)^T (d, o) -- mean folded in
            wgt = w_sb[:, 256:384]    # W_g^T  (d, o)
            c0_rep = b_sb[:, 0:128]     # c0 replicated on partitions
            bfull_rep = b_sb[:, 128:256]  # b_comb + bx
            bg_rep = b_sb[:, 256:384]   # b_g

            # ---------------- x side ----------------
            xt_sb = xpool.tile([128, T * GS], f32)
            nc.sync.dma_start(out=xt_sb[:], in_=xt[:, :])
            xg_sb = xpool.tile([128, T * GS], f32)
            for j in range(T * GS // 128):
                ps = psum.tile([128, 128], f32, tag="ps")
                nc.tensor.matmul(ps[:], lhsT=xt_sb[:, ts(j, 128)], rhs=wgt,
                                 start=True, stop=True)
                nc.vector.tensor_add(out=xg_sb[:, ts(j, 128)], in0=ps[:], in1=bg_rep)

            # xbar (sum over T; the /4 is folded into wxt)
            xu_sb = xpool.tile([128, T * nslot], f32)
            nc.sync.dma_start(out=xu_sb[:], in_=xu[:, :])
            xs01 = xpool.tile([128, nslot], f32)
            nc.vector.tensor_add(out=xs01[:], in0=xu_sb[:, 0:nslot],
                                 in1=xu_sb[:, nslot:2 * nslot])
            xs23 = xpool.tile([128, nslot], f32)
            nc.vector.tensor_add(out=xs23[:], in0=xu_sb[:, 2 * nslot:3 * nslot],
                                 in1=xu_sb[:, 3 * nslot:4 * nslot])
            xbar = xpool.tile([128, nslot], f32)
            nc.vector.tensor_add(out=xbar[:], in0=xs01[:], in1=xs23[:])

            fe_sb = xpool.tile([128, nslot], f32)
            nc.sync.dma_start(out=fe_sb[:], in_=fe[:, :])

            # slot rows: value = feats[pos]@Wc1^T + b_comb + xbar[src]@(Wx/4)^T + bx
            slot_tiles = []
            for j in range(nslot // 128):
                ps_ym = psum.tile([128, 128], f32, tag="ps")
                nc.tensor.matmul(ps_ym[:], lhsT=xbar[:, ts(j, 128)], rhs=wxt,
                                 start=True, stop=True)
                ps_fc = psum.tile([128, 128], f32, tag="ps")
                nc.tensor.matmul(ps_fc[:], lhsT=fe_sb[:, ts(j, 128)], rhs=wc1t,
                                 start=True, stop=True)
                fcb = spool.tile([128, 128], f32, tag=f"fcb{j}")
                nc.vector.tensor_add(out=fcb[:], in0=ps_fc[:], in1=bfull_rep)
                slot = spool.tile([128, 128], f32, tag=f"slot{j}")
                nc.vector.tensor_add(out=slot[:], in0=fcb[:], in1=ps_ym[:])
                slot_tiles.append(slot)

            # x_out = xg + slot[gather part], broadcast over t
            gbase = slot_s // 128
            for t in range(T):
                ox = xpool.tile([128, GS], f32, tag="ox")
                for j in range(GS // 128):
                    nc.vector.tensor_add(out=ox[:, ts(j, 128)],
                                         in0=xg_sb[:, ts(t * (GS // 128) + j, 128)],
                                         in1=slot_tiles[gbase + j][:])
                dst = out_x[t * GS:(t + 1) * GS, :].rearrange(
                    "(j p) o -> p j o", p=128)
                nc.sync.dma_start(out=dst, in_=ox[:])

            # ---------------- main dense pass ----------------
            main_out_insts = []
            for c in range(NCH):
                fin = min_pool.tile([128, CH], f32, tag="fin")
                nc.sync.dma_start(out=fin[:], in_=ft[:, c * CH:(c + 1) * CH])
                fout = mout_pool.tile([128, CH], f32, tag="fout")
                for s in range(CH // 128):
                    ps = psum.tile([128, 128], f32, tag="ps")
                    nc.tensor.matmul(ps[:], lhsT=fin[:, ts(s, 128)], rhs=wc1t,
                                     start=True, stop=True)
                    nc.vector.tensor_add(out=fout[:, ts(s, 128)], in0=ps[:],
                                         in1=c0_rep)
                dst = out_f[c * CH:(c + 1) * CH, :].rearrange(
                    "(s p) o -> p s o", p=128)
                inst = nc.sync.dma_start(out=dst, in_=fout[:])
                main_out_insts.append(inst)

            # ---------------- scatter edge rows (overwrite) ----------------
            for j in range(slot_s // 128):
                idx = spool.tile([128, 1], i32, tag=f"idx{j}")
                nc.sync.dma_start(out=idx[:], in_=sc[j * 128:(j + 1) * 128, :])
                sc_inst = nc.gpsimd.indirect_dma_start(
                    out=out_f[:, :],
                    out_offset=bass.IndirectOffsetOnAxis(ap=idx[:, :1], axis=0),
                    in_=slot_tiles[j][:],
                    in_offset=None,
                    bounds_check=JROWS - 1,
                    oob_is_err=False,
                )
                for m in main_out_insts:
                    add_dep_helper(sc_inst.ins, m.ins,
                                   reason="scatter overwrites after dense rows")

    nc.finalize()
    _BUILD_CACHE[key] = nc
    return nc


def _prep(inputs):
    """Host-side prep: weight folding (tiny) + index-only gathers/slices."""
    x = np.asarray(inputs["x"], F32)
    feats = np.asarray(inputs["feats"], F32)
    nodes = np.asarray(inputs["nodes"])
    pad = np.asarray(inputs["pad"], F32)
    W_g = np.asarray(inputs["W_g"], F32)
    b_g = np.asarray(inputs["b_g"], F32)
    W_lin = np.asarray(inputs["W_lin"], F32)
    b_lin = np.asarray(inputs["b_lin"], F32)
    W_f = np.asarray(inputs["W_f"], F32)
    b_f = np.asarray(inputs["b_f"], F32)

    Wc = (W_f @ W_lin).astype(F32)          # (D, 2D)
    Wc1, Wc2 = Wc[:, :D], Wc[:, D:]
    b_comb = (W_f @ b_lin + b_f).astype(F32)
    Wx = (Wc2 @ W_g).astype(F32)
    bx = (Wc2 @ b_g).astype(F32)
    pcv = (pad[0] * Wc2.sum(1)).astype(F32)
    c0 = (b_comb + pcv).astype(F32)
    bfull = (b_comb + bx).astype(F32)

    wcat = np.concatenate(
        [Wc1.T, (Wx.T * 0.25), W_g.T], axis=1).astype(F32)  # (128, 384)
    brep = np.concatenate(
        [np.tile(v[None, :], (128, 1)) for v in (c0, bfull, b_g)],
        axis=1).astype(F32)

    # transposed views of the big tensors (layout only)
    FT = np.ascontiguousarray(feats.transpose(3, 0, 1, 2).reshape(D, N * N * B))
    XT = np.ascontiguousarray(x.transpose(3, 0, 1, 2).reshape(D, T * E * B))

    n0 = nodes[0, :, 0].astype(np.int64) - 1
    n1 = nodes[0, :, 1].astype(np.int64) - 1
    pos = n0 * N + n1                       # (E,) in [0, N*N)
    winmap = {}
    for e in range(E):
        winmap[pos[e]] = e                  # last writer wins
    src = np.array([winmap[p] for p in pos], dtype=np.int64)

    per_core_sc = []
    for k in range(NCORES):
        lo, hi = RS * k * N, RS * (k + 1) * N
        items = sorted((p, e) for p, e in winmap.items() if lo <= p < hi)
        per_core_sc.append(items)
    max_sc = max(len(v) for v in per_core_sc) * B
    slot_s = max(128, ((max_sc + 127) // 128) * 128)
    nslot = slot_s + GS

    in_maps = []
    for k in range(NCORES):
        items = per_core_sc[k]
        nsc = len(items)
        # per-slot source arrays (slot = 2*i + b)
        fecols = np.zeros(nslot, dtype=np.int64)
        xsrc = np.zeros(nslot, dtype=np.int64)
        scidx = np.full(slot_s, 2 ** 30, dtype=np.int64)
        for i, (p, e) in enumerate(items):
            for b in range(B):
                j = 2 * i + b
                fecols[j] = p * B + b
                xsrc[j] = e
                scidx[j] = (p - RS * k * N) * B + b
        for el in range(ESH):
            e = ESH * k + el
            for b in range(B):
                j = slot_s + 2 * el + b
                fecols[j] = pos[e] * B + b
                xsrc[j] = src[e]
        bj = np.arange(nslot) % 2
        xu_blocks = [XT[:, (t * E + xsrc) * B + bj] for t in range(T)]
        xu_k = np.ascontiguousarray(np.concatenate(xu_blocks, axis=1))
        fe_k = np.ascontiguousarray(FT[:, fecols])
        ft_k = np.ascontiguousarray(FT[:, RS * k * N * B: RS * (k + 1) * N * B])
        xt_k = np.ascontiguousarray(np.concatenate(
            [XT[:, (t * E + ESH * k) * B:(t * E + ESH * (k + 1)) * B]
             for t in range(T)], axis=1))
        in_maps.append({
            "ft": ft_k,
            "xt": xt_k,
            "xu": xu_k,
            "fe": fe_k,
            "sc": scidx.astype(np.int32).reshape(slot_s, 1),
            "wcat": wcat,
            "brep": brep,
        })
    return slot_s, in_maps


def kernel(**inputs):
    from concourse import bass_utils

    slot_s, in_maps = _prep(inputs)
    nc = _build_bass(slot_s)
    res = bass_utils.run_bass_kernel_spmd(nc, in_maps,
                                          core_ids=list(range(NCORES)))
    feats_out = np.concatenate(
        [res.results[k]["out_f"].reshape(RS, N, B, D) for k in range(NCORES)],
        axis=0)
    x_out = np.concatenate(
        [res.results[k]["out_x"].reshape(T, ESH, B, D) for k in range(NCORES)],
        axis=1)
    return x_out, feats_out


# revision 4
# speedup vs baseline: 1.0596x; 1.0596x over previous
"""Trainium2 Bass kernel for nn_AxialBlock (gnn_message_passing).

Computation (see reference):
  xg    = x @ W_g^T + b_g                                  (T,E,B,D)
  xmean = xg.mean(0)                                       (E,B,D)
  grid  = pad-filled (N+1,N+1,B,D); grid[n0,n1] = xmean    (last edge wins)
  feats_out = cat(feats, grid[1:,1:]) @ W_lin^T -> @ W_f^T (N,N,B,D)
  x_out = xg + feats_out[g0,g1,bidx]                       (T,E,B,D)

Algebraic restructuring (exact, linearity of the two chained linears):
  Wc = W_f @ W_lin ; Wc1 = Wc[:,:D] ; Wc2 = Wc[:,D:]
  b_comb = W_f @ b_lin + b_f
  feats_out = feats @ Wc1^T + b_comb + (grid-term)
    - non-edge positions: grid row == pad  -> + pad * rowsum(Wc2)   (c0 bias)
    - edge positions (n0-1,n1-1):          -> + ymean[win(pos)]
  ymean = xmean @ Wc2^T = (mean_t xg) @ Wc2^T = xbar @ (Wc2 @ W_g)^T + Wc2 @ b_g
  gather rows x_out needs are exactly the edge-position rows of feats_out.

Sharding over 8 cores: feats grid rows (first N axis) 48 rows/core for the
dense pass + scatter; x is sharded over E (512 edges/core) for xg / x_out.
Edge-dependent rows are computed per-core from host-pre-gathered inputs
(pure indexing on the host; all arithmetic on device).

Device layout trick: all matmul inputs are fed pre-transposed (D on the
partition axis) so the PE consumes them directly as the stationary operand
(out rows land in natural row-major layout for contiguous DMA out).
"""

import numpy as np

T, E, B, D, N = 4, 4096, 2, 128, 384
NCORES = 8
RS = N // NCORES            # feats rows per core (48)
JROWS = RS * N * B          # dense rows per core (36864)
ESH = E // NCORES           # edges per core (512)
GS = ESH * B                # gather slots per core (1024)
F32 = np.float32

_BUILD_CACHE = {}


def _build_bass(slot_s, rep=1):
    """Build (and cache) the Bass module. slot_s = padded scatter-slot count.

    rep>1 wraps the whole body in a hardware For_i loop (timing experiments
    only; the work is idempotent so results are unchanged)."""
    key = (slot_s, rep)
    if key in _BUILD_CACHE:
        return _BUILD_CACHE[key]

    import concourse.bacc as bacc
    import concourse.mybir as mybir
    from concourse import bass
    from concourse.tile import TileContext
    from concourse.tile_rust import add_dep_helper

    f32 = mybir.dt.float32
    i32 = mybir.dt.int32
    nslot = slot_s + GS

    nc = bacc.Bacc("TRN2")
    ft = nc.dram_tensor("ft", (128, JROWS), f32, kind="ExternalInput")
    xt = nc.dram_tensor("xt", (128, T * GS), f32, kind="ExternalInput")
    xu = nc.dram_tensor("xu", (128, T * nslot), f32, kind="ExternalInput")
    fe = nc.dram_tensor("fe", (128, nslot), f32, kind="ExternalInput")
    sc = nc.dram_tensor("sc", (slot_s, 1), i32, kind="ExternalInput")
    wcat = nc.dram_tensor("wcat", (128, 3 * 128), f32, kind="ExternalInput")
    brep = nc.dram_tensor("brep", (128, 3 * 128), f32, kind="ExternalInput")
    out_f = nc.dram_tensor("out_f", (JROWS, 128), f32, kind="ExternalOutput")
    out_x = nc.dram_tensor("out_x", (T * GS, 128), f32, kind="ExternalOutput")

    ts = bass.ts
    CH = 3072               # main-pass chunk: 24 tiles of 128 rows
    NCH = JROWS // CH       # 12 chunks

    with TileContext(nc) as tc:
        with (
            tc.tile_pool(name="const", bufs=1) as cpool,
            tc.tile_pool(name="xside", bufs=1) as xpool,
            tc.tile_pool(name="slots", bufs=1) as spool,
            tc.tile_pool(name="min", bufs=3) as min_pool,
            tc.tile_pool(name="mout", bufs=3) as mout_pool,
            tc.tile_pool(name="psum", bufs=8, space="PSUM") as psum,
        ):
          from contextlib import nullcontext
          with tc.For_i(0, rep, 1) if rep > 1 else nullcontext():
            w_sb = cpool.tile([128, 384], f32)
            nc.sync.dma_start(out=w_sb[:], in_=wcat[:, :])
            b_sb = cpool.tile([128, 384], f32)
            nc.sync.dma_start(out=b_sb[:], in_=brep[:, :])
            wc1t = w_sb[:, 0:128]     # Wc1^T  (d, o)
            wxt = w_sb[:, 128:256]    # (Wx/4)^T (d, o) -- mean folded in
            wgt = w_sb[:, 256:384]    # W_g^T  (d, o)
            c0_rep = b_sb[:, 0:128]     # c0 replicated on partitions
            bfull_rep = b_sb[:, 128:256]  # b_comb + bx
            bg_rep = b_sb[:, 256:384]   # b_g

            # ---------------- x side ----------------
            xt_sb = xpool.tile([128, T * GS], f32)
            nc.sync.dma_start(out=xt_sb[:], in_=xt[:, :])
            xg_sb = xpool.tile([128, T * GS], f32)
            for j in range(T * GS // 128):
                ps = psum.tile([128, 128], f32, tag="ps")
                nc.tensor.matmul(ps[:], lhsT=xt_sb[:, ts(j, 128)], rhs=wgt,
                                 start=True, stop=True)
                nc.vector.tensor_add(out=xg_sb[:, ts(j, 128)], in0=ps[:], in1=bg_rep)

            # xbar (sum over T; the /4 is folded into wxt)
            xu_sb = xpool.tile([128, T * nslot], f32)
            nc.sync.dma_start(out=xu_sb[:], in_=xu[:, :])
            xs01 = xpool.tile([128, nslot], f32)
            nc.vector.tensor_add(out=xs01[:], in0=xu_sb[:, 0:nslot],
                                 in1=xu_sb[:, nslot:2 * nslot])
            xs23 = xpool.tile([128, nslot], f32)
            nc.vector.tensor_add(out=xs23[:], in0=xu_sb[:, 2 * nslot:3 * nslot],
                                 in1=xu_sb[:, 3 * nslot:4 * nslot])
            xbar = xpool.tile([128, nslot], f32)
            nc.vector.tensor_add(out=xbar[:], in0=xs01[:], in1=xs23[:])

            fe_sb = xpool.tile([128, nslot], f32)
            nc.sync.dma_start(out=fe_sb[:], in_=fe[:, :])

            # slot rows: value = feats[pos]@Wc1^T + b_comb + xbar[src]@(Wx/4)^T + bx
            slot_tiles = []
            for j in range(nslot // 128):
                ps_ym = psum.tile([128, 128], f32, tag="ps")
                nc.tensor.matmul(ps_ym[:], lhsT=xbar[:, ts(j, 128)], rhs=wxt,
                                 start=True, stop=True)
                ps_fc = psum.tile([128, 128], f32, tag="ps")
                nc.tensor.matmul(ps_fc[:], lhsT=fe_sb[:, ts(j, 128)], rhs=wc1t,
                                 start=True, stop=True)
                fcb = spool.tile([128, 128], f32, tag=f"fcb{j}")
                nc.vector.tensor_add(out=fcb[:], in0=ps_fc[:], in1=bfull_rep)
                slot = spool.tile([128, 128], f32, tag=f"slot{j}")
                nc.vector.tensor_add(out=slot[:], in0=fcb[:], in1=ps_ym[:])
                slot_tiles.append(slot)

            # x_out = xg + slot[gather part], broadcast over t
            gbase = slot_s // 128
            for t in range(T):
                ox = xpool.tile([128, GS], f32, tag="ox")
                for j in range(GS // 128):
                    nc.vector.tensor_add(out=ox[:, ts(j, 128)],
                                         in0=xg_sb[:, ts(t * (GS // 128) + j, 128)],
                                         in1=slot_tiles[gbase + j][:])
                dst = out_x[t * GS:(t + 1) * GS, :].rearrange(
                    "(j p) o -> p j o", p=128)
                nc.sync.dma_start(out=dst, in_=ox[:])

            # ---------------- main dense pass ----------------
            main_out_insts = []
            for c in range(NCH):
                fin = min_pool.tile([128, CH], f32, tag="fin")
                nc.sync.dma_start(out=fin[:], in_=ft[:, c * CH:(c + 1) * CH])
                fout = mout_pool.tile([128, CH], f32, tag="fout")
                for s in range(CH // 128):
                    ps = psum.tile([128, 128], f32, tag="ps")
                    nc.tensor.matmul(ps[:], lhsT=fin[:, ts(s, 128)], rhs=wc1t,
                                     start=True, stop=True)
                    nc.vector.tensor_add(out=fout[:, ts(s, 128)], in0=ps[:],
                                         in1=c0_rep)
                dst = out_f[c * CH:(c + 1) * CH, :].rearrange(
                    "(s p) o -> p s o", p=128)
                inst = nc.sync.dma_start(out=dst, in_=fout[:])
                main_out_insts.append(inst)

            # ---------------- scatter edge rows (overwrite) ----------------
            for j in range(slot_s // 128):
                idx = spool.tile([128, 1], i32, tag=f"idx{j}")
                nc.sync.dma_start(out=idx[:], in_=sc[j * 128:(j + 1) * 128, :])
                sc_inst = nc.gpsimd.indirect_dma_start(
                    out=out_f[:, :],
                    out_offset=bass.IndirectOffsetOnAxis(ap=idx[:, :1], axis=0),
                    in_=slot_tiles[j][:],
                    in_offset=None,
                    bounds_check=JROWS - 1,
                    oob_is_err=False,
                )
                for m in main_out_insts:
                    add_dep_helper(sc_inst.ins, m.ins,
                                   reason="scatter overwrites after dense rows")

    nc.finalize()
    _BUILD_CACHE[key] = nc
    return nc


def _prep(inputs):
    """Host-side prep: weight folding (tiny) + index-only gathers/slices."""
    x = np.asarray(inputs["x"], F32)
    feats = np.asarray(inputs["feats"], F32)
    nodes = np.asarray(inputs["nodes"])
    pad = np.asarray(inputs["pad"], F32)
    W_g = np.asarray(inputs["W_g"], F32)
    b_g = np.asarray(inputs["b_g"], F32)
    W_lin = np.asarray(inputs["W_lin"], F32)
    b_lin = np.asarray(inputs["b_lin"], F32)
    W_f = np.asarray(inputs["W_f"], F32)
    b_f = np.asarray(inputs["b_f"], F32)

    Wc = (W_f @ W_lin).astype(F32)          # (D, 2D)
    Wc1, Wc2 = Wc[:, :D], Wc[:, D:]
    b_comb = (W_f @ b_lin + b_f).astype(F32)
    Wx = (Wc2 @ W_g).astype(F32)
    bx = (Wc2 @ b_g).astype(F32)
    pcv = (pad[0] * Wc2.sum(1)).astype(F32)
    c0 = (b_comb + pcv).astype(F32)
    bfull = (b_comb + bx).astype(F32)

    wcat = np.concatenate(
        [Wc1.T, (Wx.T * 0.25), W_g.T], axis=1).astype(F32)  # (128, 384)
    brep = np.concatenate(
        [np.tile(v[None, :], (128, 1)) for v in (c0, bfull, b_g)],
        axis=1).astype(F32)

    # transposed views of the big tensors (layout only)
    FT = np.ascontiguousarray(feats.transpose(3, 0, 1, 2).reshape(D, N * N * B))
    XT = np.ascontiguousarray(x.transpose(3, 0, 1, 2).reshape(D, T * E * B))

    n0 = nodes[0, :, 0].astype(np.int64) - 1
    n1 = nodes[0, :, 1].astype(np.int64) - 1
    pos = n0 * N + n1                       # (E,) in [0, N*N)
    winmap = {}
    for e in range(E):
        winmap[pos[e]] = e                  # last writer wins
    src = np.array([winmap[p] for p in pos], dtype=np.int64)

    per_core_sc = []
    for k in range(NCORES):
        lo, hi = RS * k * N, RS * (k + 1) * N
        items = sorted((p, e) for p, e in winmap.items() if lo <= p < hi)
        per_core_sc.append(items)
    max_sc = max(len(v) for v in per_core_sc) * B
    slot_s = max(128, ((max_sc + 127) // 128) * 128)
    nslot = slot_s + GS

    in_maps = []
    for k in range(NCORES):
        items = per_core_sc[k]
        nsc = len(items)
        # per-slot source arrays (slot = 2*i + b)
        fecols = np.zeros(nslot, dtype=np.int64)
        xsrc = np.zeros(nslot, dtype=np.int64)
        # padded slots point one row past the end -> skipped by bounds_check
        # (kept small so index*row_bytes can't overflow int32 anywhere)
        scidx = np.full(slot_s, JROWS, dtype=np.int64)
        for i, (p, e) in enumerate(items):
            for b in range(B):
                j = 2 * i + b
                fecols[j] = p * B + b
                xsrc[j] = e
                scidx[j] = (p - RS * k * N) * B + b
        for el in range(ESH):
            e = ESH * k + el
            for b in range(B):
                j = slot_s + 2 * el + b
                fecols[j] = pos[e] * B + b
                xsrc[j] = src[e]
        bj = np.arange(nslot) % 2
        xu_blocks = [XT[:, (t * E + xsrc) * B + bj] for t in range(T)]
        xu_k = np.ascontiguousarray(np.concatenate(xu_blocks, axis=1))
        fe_k = np.ascontiguousarray(FT[:, fecols])
        ft_k = np.ascontiguousarray(FT[:, RS * k * N * B: RS * (k + 1) * N * B])
        xt_k = np.ascontiguousarray(np.concatenate(
            [XT[:, (t * E + ESH * k) * B:(t * E + ESH * (k + 1)) * B]
             for t in range(T)], axis=1))
        in_maps.append({
            "ft": ft_k,
            "xt": xt_k,
            "xu": xu_k,
            "fe": fe_k,
            "sc": scidx.astype(np.int32).reshape(slot_s, 1),
            "wcat": wcat,
            "brep": brep,
        })
    return slot_s, in_maps


def kernel(**inputs):
    from concourse import bass_utils

    slot_s, in_maps = _prep(inputs)
    nc = _build_bass(slot_s)
    res = bass_utils.run_bass_kernel_spmd(nc, in_maps,
                                          core_ids=list(range(NCORES)))
    feats_out = np.concatenate(
        [res.results[k]["out_f"].reshape(RS, N, B, D) for k in range(NCORES)],
        axis=0)
    x_out = np.concatenate(
        [res.results[k]["out_x"].reshape(T, ESH, B, D) for k in range(NCORES)],
        axis=1)
    return x_out, feats_out


# revision 14
# speedup vs baseline: 1.4382x; 1.3573x over previous
"""Trainium2 Bass kernel for nn_AxialBlock (gnn_message_passing).

Computation (see reference):
  xg    = x @ W_g^T + b_g                                  (T,E,B,D)
  xmean = xg.mean(0)                                       (E,B,D)
  grid  = pad-filled (N+1,N+1,B,D); grid[n0,n1] = xmean    (last edge wins)
  feats_out = cat(feats, grid[1:,1:]) @ W_lin^T -> @ W_f^T (N,N,B,D)
  x_out = xg + feats_out[g0,g1,bidx]                       (T,E,B,D)

Algebraic restructuring (exact, linearity of the two chained linears):
  Wc = W_f @ W_lin ; Wc1 = Wc[:,:D] ; Wc2 = Wc[:,D:]
  b_comb = W_f @ b_lin + b_f
  feats_out = feats @ Wc1^T + b_comb + (grid-term)
    - non-edge positions: grid row == pad  -> + pad * rowsum(Wc2)   (c0 bias)
    - edge positions (n0-1,n1-1):          -> + ymean[win(pos)]
  ymean = xmean @ Wc2^T = (mean_t xg) @ Wc2^T = xbar @ (Wc2 @ W_g)^T + Wc2 @ b_g
  gather rows x_out needs are exactly the edge-position rows of feats_out.

Sharding over 8 cores: feats grid rows (first N axis) 48 rows/core for the
dense pass + scatter; x is sharded over E (512 edges/core) for xg / x_out.
Edge-dependent rows are computed per-core from host-pre-gathered inputs
(pure indexing on the host; all arithmetic on device).

Device layout trick: all matmul inputs are fed pre-transposed (D on the
partition axis) so the PE consumes them directly as the stationary operand
(out rows land in natural row-major layout for contiguous DMA out).
"""

import numpy as np

T, E, B, D, N = 4, 4096, 2, 128, 384
NCORES = 8
RS = N // NCORES            # feats rows per core (48)
JROWS = RS * N * B          # dense rows per core (36864)
ESH = E // NCORES           # edges per core (512)
GS = ESH * B                # gather slots per core (1024)
F32 = np.float32

_BUILD_CACHE = {}


def _build_bass(slot_s, rep=1, sc_deps=None):
    """Build (and cache) the Bass module. slot_s = padded scatter-slot count.

    sc_deps: per scatter-tile tuple of main-chunk indices whose output rows
    the tile may overwrite (union over cores); None = depend on all chunks.
    rep>1 wraps the whole body in a hardware For_i loop (timing experiments
    only; the work is idempotent so results are unchanged)."""
    key = (slot_s, rep, sc_deps)
    if key in _BUILD_CACHE:
        return _BUILD_CACHE[key]

    import concourse.bacc as bacc
    import concourse.mybir as mybir
    from concourse import bass
    from concourse.tile import TileContext
    from concourse.tile_rust import add_dep_helper

    f32 = mybir.dt.float32
    i32 = mybir.dt.int32
    nslot = slot_s + GS

    nc = bacc.Bacc("TRN2")
    ft = nc.dram_tensor("ft", (128, JROWS), f32, kind="ExternalInput")
    xt = nc.dram_tensor("xt", (128, T * GS), f32, kind="ExternalInput")
    xu = nc.dram_tensor("xu", (128, T * nslot), f32, kind="ExternalInput")
    fe = nc.dram_tensor("fe", (128, nslot), f32, kind="ExternalInput")
    sc = nc.dram_tensor("sc", (slot_s, 1), i32, kind="ExternalInput")
    wcat = nc.dram_tensor("wcat", (128, 3 * 128), f32, kind="ExternalInput")
    brep = nc.dram_tensor("brep", (128, 1152), f32, kind="ExternalInput")
    out_f = nc.dram_tensor("out_f", (JROWS, 128), f32, kind="ExternalOutput")
    out_x = nc.dram_tensor("out_x", (T * GS, 128), f32, kind="ExternalOutput")

    ts = bass.ts
    CH = 3072               # main-pass chunk: 24 tiles of 128 rows
    NCH = JROWS // CH       # 12 chunks

    with TileContext(nc) as tc:
        with (
            tc.tile_pool(name="const", bufs=1) as cpool,
            tc.tile_pool(name="xside", bufs=1) as xpool,
            tc.tile_pool(name="slots", bufs=1) as spool,
            tc.tile_pool(name="min", bufs=3) as min_pool,
            tc.tile_pool(name="mout", bufs=3) as mout_pool,
            tc.tile_pool(name="psum", bufs=8, space="PSUM") as psum,
        ):
          from contextlib import nullcontext
          with tc.For_i(0, rep, 1) if rep > 1 else nullcontext():
            w_sb = cpool.tile([128, 384], f32)
            nc.sync.dma_start(out=w_sb[:], in_=wcat[:, :])
            b_sb = cpool.tile([128, 1152], f32)
            nc.sync.dma_start(out=b_sb[:], in_=brep[:, :])
            wc1t = w_sb[:, 0:128]     # Wc1^T  (d, o)
            wxt = w_sb[:, 128:256]    # (Wx/4)^T (d, o) -- mean folded in
            wgt = w_sb[:, 256:384]    # W_g^T  (d, o)
            c0_rep4 = b_sb[:, 0:512]     # c0 replicated, tiled 4x along free
            bg_rep4 = b_sb[:, 512:1024]  # b_g tiled 4x
            bfull_rep = b_sb[:, 1024:1152]  # b_comb + bx

            # ---------------- x side ----------------
            xt_sb = xpool.tile([128, T * GS], f32)
            nc.sync.dma_start(out=xt_sb[:], in_=xt[:, :])
            xg_sb = xpool.tile([128, T * GS], f32)
            for g in range(T * GS // 512):
                psb = psum.tile([128, 512], f32, tag="ps")
                for s in range(4):
                    nc.tensor.matmul(psb[:, ts(s, 128)],
                                     lhsT=xt_sb[:, ts(4 * g + s, 128)], rhs=wgt,
                                     start=True, stop=True)
                nc.vector.tensor_add(out=xg_sb[:, ts(g, 512)], in0=psb[:],
                                     in1=bg_rep4)

            # xbar (sum over T; the /4 is folded into wxt)
            xu_sb = xpool.tile([128, T * nslot], f32)
            nc.scalar.dma_start(out=xu_sb[:], in_=xu[:, :])
            xs01 = xpool.tile([128, nslot], f32)
            nc.vector.tensor_add(out=xs01[:], in0=xu_sb[:, 0:nslot],
                                 in1=xu_sb[:, nslot:2 * nslot])
            xs23 = xpool.tile([128, nslot], f32)
            nc.vector.tensor_add(out=xs23[:], in0=xu_sb[:, 2 * nslot:3 * nslot],
                                 in1=xu_sb[:, 3 * nslot:4 * nslot])
            xbar = xpool.tile([128, nslot], f32)
            nc.vector.tensor_add(out=xbar[:], in0=xs01[:], in1=xs23[:])

            fe_sb = xpool.tile([128, nslot], f32)
            nc.scalar.dma_start(out=fe_sb[:], in_=fe[:, :])

            # slot rows: value = feats[pos]@Wc1^T + b_comb + xbar[src]@(Wx/4)^T + bx
            slot_tiles = []
            for j in range(nslot // 128):
                ps_ym = psum.tile([128, 128], f32, tag="ps")
                nc.tensor.matmul(ps_ym[:], lhsT=xbar[:, ts(j, 128)], rhs=wxt,
                                 start=True, stop=True)
                ps_fc = psum.tile([128, 128], f32, tag="ps")
                nc.tensor.matmul(ps_fc[:], lhsT=fe_sb[:, ts(j, 128)], rhs=wc1t,
                                 start=True, stop=True)
                fcb = spool.tile([128, 128], f32, tag=f"fcb{j}")
                nc.vector.tensor_add(out=fcb[:], in0=ps_fc[:], in1=bfull_rep)
                slot = spool.tile([128, 128], f32, tag=f"slot{j}")
                nc.vector.tensor_add(out=slot[:], in0=fcb[:], in1=ps_ym[:])
                slot_tiles.append(slot)

            # x_out = xg + slot[gather part], broadcast over t
            gbase = slot_s // 128
            for t in range(T):
                ox = xpool.tile([128, GS], f32, tag="ox")
                for j in range(GS // 128):
                    nc.vector.tensor_add(out=ox[:, ts(j, 128)],
                                         in0=xg_sb[:, ts(t * (GS // 128) + j, 128)],
                                         in1=slot_tiles[gbase + j][:])
                dst = out_x[t * GS:(t + 1) * GS, :].rearrange(
                    "(j p) o -> p j o", p=128)
                nc.sync.dma_start(out=dst, in_=ox[:])

            # ---------------- main dense pass ----------------
            main_out_insts = []
            for c in range(NCH):
                eng = nc.sync if c % 2 == 0 else nc.scalar
                fin = min_pool.tile([128, CH], f32, tag="fin")
                eng.dma_start(out=fin[:], in_=ft[:, c * CH:(c + 1) * CH])
                fout = mout_pool.tile([128, CH], f32, tag="fout")
                for g in range(CH // 512):
                    psb = psum.tile([128, 512], f32, tag="ps")
                    for s in range(4):
                        nc.tensor.matmul(psb[:, ts(s, 128)],
                                         lhsT=fin[:, ts(4 * g + s, 128)],
                                         rhs=wc1t, start=True, stop=True)
                    nc.vector.tensor_add(out=fout[:, ts(g, 512)], in0=psb[:],
                                         in1=c0_rep4)
                dst = out_f[c * CH:(c + 1) * CH, :].rearrange(
                    "(s p) o -> p s o", p=128)
                inst = eng.dma_start(out=dst, in_=fout[:])
                main_out_insts.append(inst)

            # ---------------- scatter edge rows (overwrite) ----------------
            for j in range(slot_s // 128):
                idx = spool.tile([128, 1], i32, tag=f"idx{j}")
                nc.sync.dma_start(out=idx[:], in_=sc[j * 128:(j + 1) * 128, :])
                sc_inst = nc.gpsimd.indirect_dma_start(
                    out=out_f[:, :],
                    out_offset=bass.IndirectOffsetOnAxis(ap=idx[:, :1], axis=0),
                    in_=slot_tiles[j][:],
                    in_offset=None,
                    bounds_check=JROWS - 1,
                    oob_is_err=False,
                )
                deps = range(NCH) if sc_deps is None else sc_deps[j]
                for c in deps:
                    add_dep_helper(sc_inst.ins, main_out_insts[c].ins,
                                   reason="scatter overwrites after dense rows")

    nc.finalize()
    _BUILD_CACHE[key] = nc
    return nc


def _prep(inputs):
    """Host-side prep: weight folding (tiny) + index-only gathers/slices."""
    x = np.asarray(inputs["x"], F32)
    feats = np.asarray(inputs["feats"], F32)
    nodes = np.asarray(inputs["nodes"])
    pad = np.asarray(inputs["pad"], F32)
    W_g = np.asarray(inputs["W_g"], F32)
    b_g = np.asarray(inputs["b_g"], F32)
    W_lin = np.asarray(inputs["W_lin"], F32)
    b_lin = np.asarray(inputs["b_lin"], F32)
    W_f = np.asarray(inputs["W_f"], F32)
    b_f = np.asarray(inputs["b_f"], F32)

    Wc = (W_f @ W_lin).astype(F32)          # (D, 2D)
    Wc1, Wc2 = Wc[:, :D], Wc[:, D:]
    b_comb = (W_f @ b_lin + b_f).astype(F32)
    Wx = (Wc2 @ W_g).astype(F32)
    bx = (Wc2 @ b_g).astype(F32)
    pcv = (pad[0] * Wc2.sum(1)).astype(F32)
    c0 = (b_comb + pcv).astype(F32)
    bfull = (b_comb + bx).astype(F32)

    wcat = np.concatenate(
        [Wc1.T, (Wx.T * 0.25), W_g.T], axis=1).astype(F32)  # (128, 384)
    brep = np.concatenate(
        [np.tile(c0[None, :], (128, 4)), np.tile(b_g[None, :], (128, 4)),
         np.tile(bfull[None, :], (128, 1))], axis=1).astype(F32)  # (128,1152)

    # transposed views of the big tensors (layout only)
    FT = np.ascontiguousarray(feats.transpose(3, 0, 1, 2).reshape(D, N * N * B))
    XT = np.ascontiguousarray(x.transpose(3, 0, 1, 2).reshape(D, T * E * B))

    n0 = nodes[0, :, 0].astype(np.int64) - 1
    n1 = nodes[0, :, 1].astype(np.int64) - 1
    pos = n0 * N + n1                       # (E,) in [0, N*N)
    winmap = {}
    for e in range(E):
        winmap[pos[e]] = e                  # last writer wins
    src = np.array([winmap[p] for p in pos], dtype=np.int64)

    per_core_sc = []
    for k in range(NCORES):
        lo, hi = RS * k * N, RS * (k + 1) * N
        items = sorted((p, e) for p, e in winmap.items() if lo <= p < hi)
        per_core_sc.append(items)
    max_sc = max(len(v) for v in per_core_sc) * B
    slot_s = max(128, ((max_sc + 127) // 128) * 128)
    nslot = slot_s + GS

    # per scatter-tile set of main chunks (of 3072 rows) it can overwrite,
    # unioned over cores (one SPMD program for all cores)
    CH = 3072
    dep_sets = [set() for _ in range(slot_s // 128)]
    for k in range(NCORES):
        for i, (p, e) in enumerate(per_core_sc[k]):
            for b in range(B):
                j = 2 * i + b
                row = (p - RS * k * N) * B + b
                dep_sets[j // 128].add(row // CH)
    sc_deps = tuple(tuple(sorted(s)) if s else (0,) for s in dep_sets)

    in_maps = []
    for k in range(NCORES):
        items = per_core_sc[k]
        nsc = len(items)
        # per-slot source arrays (slot = 2*i + b)
        fecols = np.zeros(nslot, dtype=np.int64)
        xsrc = np.zeros(nslot, dtype=np.int64)
        # padded slots point one row past the end -> skipped by bounds_check
        # (kept small so index*row_bytes can't overflow int32 anywhere)
        scidx = np.full(slot_s, JROWS, dtype=np.int64)
        for i, (p, e) in enumerate(items):
            for b in range(B):
                j = 2 * i + b
                fecols[j] = p * B + b
                xsrc[j] = e
                scidx[j] = (p - RS * k * N) * B + b
        for el in range(ESH):
            e = ESH * k + el
            for b in range(B):
                j = slot_s + 2 * el + b
                fecols[j] = pos[e] * B + b
                xsrc[j] = src[e]
        bj = np.arange(nslot) % 2
        xu_blocks = [XT[:, (t * E + xsrc) * B + bj] for t in range(T)]
        xu_k = np.ascontiguousarray(np.concatenate(xu_blocks, axis=1))
        fe_k = np.ascontiguousarray(FT[:, fecols])
        ft_k = np.ascontiguousarray(FT[:, RS * k * N * B: RS * (k + 1) * N * B])
        xt_k = np.ascontiguousarray(np.concatenate(
            [XT[:, (t * E + ESH * k) * B:(t * E + ESH * (k + 1)) * B]
             for t in range(T)], axis=1))
        in_maps.append({
            "ft": ft_k,
            "xt": xt_k,
            "xu": xu_k,
            "fe": fe_k,
            "sc": scidx.astype(np.int32).reshape(slot_s, 1),
            "wcat": wcat,
            "brep": brep,
        })
    return slot_s, in_maps, sc_deps


def kernel(**inputs):
    from concourse import bass_utils

    slot_s, in_maps, sc_deps = _prep(inputs)
    nc = _build_bass(slot_s, sc_deps=sc_deps)
    res = bass_utils.run_bass_kernel_spmd(nc, in_maps,
                                          core_ids=list(range(NCORES)))
    feats_out = np.concatenate(
        [res.results[k]["out_f"].reshape(RS, N, B, D) for k in range(NCORES)],
        axis=0)
    x_out = np.concatenate(
        [res.results[k]["out_x"].reshape(T, ESH, B, D) for k in range(NCORES)],
        axis=1)
    return x_out, feats_out
